# revision 1
# baseline (speedup 1.0000x reference)
"""Trainium2 Bass kernel for the 4-modality attention-fusion module.

Computes, for full inputs mod0..mod3 [16384, 1024] f32 and W [1024, 1024] f32:
    scores_m = mod_m @ W.T                      (per modality)
    attn     = softmax over m of scores         (elementwise over [B, L])
    fused    = sum_m mod_m * attn_m
    scaler_b = 1 + #{m : sum_l mod_m[b, l] == 0}
    out      = fused * scaler[:, None]

Sharded data-parallel over 8 NeuronCores along the batch dim (2048 rows each),
W replicated. Per core, per 128-patient tile:
  - PE transposes mod blocks ([128,128]) into the matmul stationaries
    (contraction over l needs l on partitions); ACT evicts them from PSUM
    (rounding to float32r), keeping the DVE free;
  - float32r matmuls against a resident W.T accumulate scores in PSUM;
  - row sums (zero-modality detection) ride N=1 ones-matmuls on the same
    stationaries;
  - softmax over the 4 modalities: exp on ACT straight out of PSUM,
    denominator adds on GPSIMD (no port conflict with fp32 tensor_tensor),
    products/sums/reciprocal on DVE with in-place tile reuse,
    zero-modality rescale folded into the reciprocal;
  - emission is software-pipelined: each segment issues PE work for tile p
    interleaved with exps of tile p (freeing PSUM) and the trailing
    DVE/GPSIMD chain of tile p-1, so the PE never idles long enough to
    re-throttle its clock.
"""

import sys

sys.path.insert(0, "/opt/trn_rl_repo")

from contextlib import ExitStack

import numpy as np

import concourse.bass as bass
import concourse.bacc as bacc
import concourse.mybir as mybir
import concourse.tile as tile
from concourse.bass_utils import run_bass_kernel_spmd
from concourse.masks import make_identity

F32 = mybir.dt.float32
F32R = mybir.dt.float32r
AF = mybir.ActivationFunctionType

N_CORES = 8
B_FULL = 16384
L = 1024
P = 128
B_SHARD = B_FULL // N_CORES          # 2048
NPT = B_SHARD // P                   # 16 patient tiles per core
NM = 4                               # modalities
NLC = L // P                         # 8 l-chunks (contraction)
NH = 2                               # k halves
KH = L // NH                         # 512

MM_DT = F32R

_CACHE: dict = {}


def _build(
    repeat: int = 1,
    *,
    mains: bool = True,
    rowsums: bool = True,
    elem: bool = True,
    adds_engine: str = "dve",
    dve_chunk: int = 0,
):
    nc = bacc.Bacc("TRN2", target_bir_lowering=False, debug=False)
    mods_d = [
        nc.dram_tensor(f"mod{m}", [B_SHARD, L], F32, kind="ExternalInput").ap()
        for m in range(NM)
    ]
    w_d = nc.dram_tensor("W", [L, L], F32, kind="ExternalInput").ap()
    out_d = nc.dram_tensor("out", [B_SHARD, L], F32, kind="ExternalOutput").ap()

    with tile.TileContext(nc) as tc, ExitStack() as ctx:
        const_p = ctx.enter_context(tc.tile_pool(name="const", bufs=1))
        wt_p = ctx.enter_context(tc.tile_pool(name="wt", bufs=1))
        wload_p = ctx.enter_context(tc.tile_pool(name="wload", bufs=1))
        mod_p = ctx.enter_context(tc.tile_pool(name="mod", bufs=4))
        modt_p = ctx.enter_context(tc.tile_pool(name="modt", bufs=2))
        e_p = ctx.enter_context(tc.tile_pool(name="e", bufs=4))
        tmp_p = ctx.enter_context(tc.tile_pool(name="tmp", bufs=4))
        out_p = ctx.enter_context(tc.tile_pool(name="outp", bufs=2))
        ps_t = ctx.enter_context(
            tc.tile_pool(name="ps_t", bufs=2, space=bass.MemorySpace.PSUM)
        )
        ps_q = ctx.enter_context(
            tc.tile_pool(name="ps_q", bufs=6, space=bass.MemorySpace.PSUM)
        )

        ident = const_p.tile([P, P], F32, tag="ident")
        make_identity(nc, ident[:])
        ones8f = const_p.tile([P, 8], F32, tag="ones8f")
        nc.vector.memset(ones8f[:], 1.0)
        ones8 = const_p.tile([P, 8], MM_DT, tag="ones8")
        nc.vector.tensor_copy(ones8[:], ones8f[:])

        # ---- Build WT resident in SBUF: wt[p, lc, k] = W[k, lc*128 + p] ----
        wt = wt_p.tile([P, NLC, L], MM_DT, tag="wt")
        for kc in range(NLC):
            wk = wload_p.tile([P, L], F32, tag="wk")
            nc.sync.dma_start(wk[:], w_d[kc * P : (kc + 1) * P, :])
            for g in range(2):
                pt = ps_t.tile([P, 4 * P], F32, tag="pt")
                for j in range(4):
                    lc = g * 4 + j
                    nc.tensor.transpose(
                        pt[:, j * P : (j + 1) * P],
                        wk[:, lc * P : (lc + 1) * P],
                        ident[:],
                    )
                # block j holds l-chunk g*4+j of this k block
                nc.scalar.copy(
                    wt[:, g * 4 : (g + 1) * 4, kc * P : (kc + 1) * P], pt[:]
                )

        # ---------------- main loop, software-pipelined ----------------
        rep_cm = (
            tc.For_i(
                0,
                repeat,
                1,
                hint_engines=(
                    mybir.EngineType.PE,
                    mybir.EngineType.DVE,
                    mybir.EngineType.Activation,
                    mybir.EngineType.Pool,
                    mybir.EngineType.SP,
                ),
            )
            if repeat > 1
            else None
        )
        if rep_cm is not None:
            rep_cm.__enter__()

        def emit_mod_dma(p):
            row = slice(p * P, (p + 1) * P)
            mods = []
            for m in range(NM):
                mt = mod_p.tile([P, L], F32, tag=f"mod{m}")
                nc.sync.dma_start(mt[:], mods_d[m][row, :])
                mods.append(mt)
            return mods

        def emit_pe_segment(p, mods):
            """Transposes + evicts + score matmuls + exps for tile p."""
            es = {}
            modTs = []

            def emit_transp(m):
                mT = modt_p.tile([P, NLC, P], MM_DT, tag=f"modt{m}")
                for g in range(2):
                    pt = ps_t.tile([P, 4 * P], F32, tag="pt")
                    for j in range(4):
                        lc = g * 4 + j
                        nc.tensor.transpose(
                            pt[:, j * P : (j + 1) * P],
                            mods[m][:, lc * P : (lc + 1) * P],
                            ident[:],
                        )
                    nc.scalar.copy(mT[:, g * 4 : (g + 1) * 4, :], pt[:])
                modTs.append(mT)

            def emit_mains(m, psums):
                sqs = []
                for _h in range(NH):
                    sq = ps_q.tile([P, KH], F32, tag="sq")
                    sqs.append(sq)
                if mains:
                    # interleave the two k-half chains and the rowsum chain so
                    # each stationary modT[m][lc] feeds 3 consecutive matmuls
                    for lc in range(NLC):
                        for h in range(NH):
                            nc.tensor.matmul(
                                sqs[h][:],
                                modTs[m][:, lc, :],
                                wt[:, lc, h * KH : (h + 1) * KH],
                                start=(lc == 0),
                                stop=(lc == NLC - 1),
                            )
                else:
                    for h in range(NH):
                        nc.tensor.matmul(
                            sqs[h][:], wt[:, 0, 0:P], wt[:, 0, h * KH : (h + 1) * KH],
                            start=True, stop=True,
                        )
                return sqs

            def emit_exps(m, sqs):
                for h in range(NH):
                    e = e_p.tile([P, KH], F32, tag=f"e{m}")
                    nc.scalar.activation(e[:], sqs[h][:], AF.Exp)
                    es[(m, h)] = e

            # transposes lead the score matmuls by one modality so the
            # ACT evictions are done before the PE needs the stationaries;
            # exps trail one modality so they don't block later evictions
            # in the ACT queue.
            psums = None
            if rowsums:
                # row sums on ACT: Copy with accum_out reduces along the free
                # dim of the natural mod tile = sum over l, in full fp32.
                # Output goes to a scratch tile (not in place) so the PE
                # transposes reading mod are not serialized behind a write.
                psums = tmp_p.tile([P, NM], F32, tag="psums")
            emit_transp(0)
            emit_transp(1)
            for m in range(NM):
                sqs_m = emit_mains(m, psums)
                if m + 2 < NM:
                    emit_transp(m + 2)
                if elem:
                    emit_exps(m, sqs_m)

            if psums is not None:
                scratch = tmp_p.tile([P, L], F32, tag="scratch")
                for m in range(NM):
                    nc.scalar.activation(
                        scratch[:],
                        mods[m][:],
                        AF.Copy,
                        accum_out=psums[:, m : m + 1],
                    )

            return p, mods, es, psums

        def emit_tail(state):
            """GPSIMD/DVE softmax chain + output for tile p (lags one segment).
            psums(p) completed long ago, so the scaler ops don't stall the
            DVE queue; independent muls run while GPSIMD builds the den."""
            p, mods, es, psums = state
            row = slice(p * P, (p + 1) * P)
            scaler = None
            if psums is not None:
                zt = tmp_p.tile([P, NM], F32, tag="zt")
                zs = tmp_p.tile([P, 1], F32, tag="zs")
                nc.vector.tensor_scalar(
                    out=zt[:],
                    in0=psums[:],
                    scalar1=0.0,
                    scalar2=None,
                    op0=mybir.AluOpType.is_equal,
                    op1=mybir.AluOpType.add,
                    accum_out=zs[:],
                )
                scaler = tmp_p.tile([P, 1], F32, tag="scaler")
                nc.vector.tensor_scalar_add(scaler[:], zs[:], 1.0)
            ot = out_p.tile([P, L], F32, tag="ot")
            for h in range(NH):
                hs = slice(h * KH, (h + 1) * KH)
                e0, e1, e2, e3 = (es[(m, h)] for m in range(NM))
                # denominator on GPSIMD (no port clash with fp32 TT on DVE)
                d01 = tmp_p.tile([P, KH], F32, tag="d01")
                d23 = tmp_p.tile([P, KH], F32, tag="d23")
                adde = nc.gpsimd if adds_engine == "gpsimd" else nc.vector
                adde.tensor_add(d01[:], e0[:], e1[:])
                adde.tensor_add(d23[:], e2[:], e3[:])
                adde.tensor_add(d01[:], d01[:], d23[:])
                # numerator products first on DVE — independent of the den,
                # so the DVE isn't head-of-line blocked waiting on GPSIMD
                C = dve_chunk if dve_chunk else KH
                for m in range(NM):
                    for c0 in range(0, KH, C):
                        cs = slice(c0, c0 + C)
                        nc.vector.tensor_mul(
                            es[(m, h)][:, cs],
                            es[(m, h)][:, cs],
                            mods[m][:, h * KH + c0 : h * KH + c0 + C],
                        )
                for c0 in range(0, KH, C):
                    cs = slice(c0, c0 + C)
                    nc.vector.tensor_add(e0[:, cs], e0[:, cs], e1[:, cs])
                nc.gpsimd.tensor_add(e2[:], e2[:], e3[:])
                # r = 1/den (in place)
                for c0 in range(0, KH, C):
                    cs = slice(c0, c0 + C)
                    nc.vector.reciprocal_approx_fast(
                        out=d01[:, cs], in_=d01[:, cs]
                    )
                for c0 in range(0, KH, C):
                    cs = slice(c0, c0 + C)
                    nc.vector.tensor_add(e0[:, cs], e0[:, cs], e2[:, cs])
                # ot = (r * scaler) * num in one DVE op
                for c0 in range(0, KH, C):
                    cs = slice(c0, c0 + C)
                    nc.vector.scalar_tensor_tensor(
                        out=ot[:, h * KH + c0 : h * KH + c0 + C],
                        in0=d01[:, cs],
                        scalar=scaler[:] if scaler is not None else 1.0,
                        in1=e0[:, cs],
                        op0=mybir.AluOpType.mult,
                        op1=mybir.AluOpType.mult,
                    )
            nc.sync.dma_start(out_d[row, :], ot[:])

        prev = None
        mods_next = emit_mod_dma(0)
        for p in range(NPT):
            mods_cur = mods_next
            state = emit_pe_segment(p, mods_cur)
            if p + 1 < NPT:
                mods_next = emit_mod_dma(p + 1)
            if prev is not None and elem:
                emit_tail(prev)
            prev = state
        if elem:
            emit_tail(prev)

        if rep_cm is not None:
            rep_cm.__exit__(None, None, None)

    nc.compile()
    return nc


def _get_nc(repeat: int = 1, **flags):
    key = ("nc", repeat, tuple(sorted(flags.items())))
    if key not in _CACHE:
        _CACHE[key] = _build(repeat, **flags)
    return _CACHE[key]


def _run(inputs, trace=False):
    nc = _get_nc()
    w = np.ascontiguousarray(np.asarray(inputs["W"], dtype=np.float32))
    in_maps = []
    for c in range(N_CORES):
        sl = slice(c * B_SHARD, (c + 1) * B_SHARD)
        im = {"W": w}
        for m in range(NM):
            im[f"mod{m}"] = np.ascontiguousarray(
                np.asarray(inputs[f"mod{m}"], dtype=np.float32)[sl]
            )
        in_maps.append(im)
    return run_bass_kernel_spmd(
        nc, in_maps, core_ids=list(range(N_CORES)), trace=trace
    )


def kernel(**inputs) -> np.ndarray:
    res = _run(inputs, trace=False)
    return np.concatenate(
        [res.results[c]["out"] for c in range(N_CORES)], axis=0
    ).astype(np.float32)



# revision 3
# speedup vs baseline: 633.2159x; 633.2159x over previous
"""Trainium2 Bass kernel for the 4-modality attention-fusion module.

Computes, for full inputs mod0..mod3 [16384, 1024] f32 and W [1024, 1024] f32:
    scores_m = mod_m @ W.T                      (per modality)
    attn     = softmax over m of scores         (elementwise over [B, L])
    fused    = sum_m mod_m * attn_m
    scaler_b = 1 + #{m : sum_l mod_m[b, l] == 0}
    out      = fused * scaler[:, None]

Sharded data-parallel over 8 NeuronCores along the batch dim (2048 rows each),
W replicated. Design (v2 — PE runs ONLY the score matmuls):
  - W.T is built resident in SBUF in bf16 once, via XBAR DMA transposes
    (wt[p, j, k] = W[k, j*128 + p]);
  - per 128-patient tile, the f32 mod tiles are cast to bf16 on ACT; that
    same ACT op carries accum_out, producing the per-row sums (zero-modality
    detection) for free;
  - the bf16 mod tiles are transposed by the DMA XBAR engine (SBUF->SBUF,
    ~0.9us per [128,1024]) into the matmul stationary layout — the PE does
    no transposes and the ACT no PSUM evictions;
  - bf16 matmuls accumulate scores in PSUM (1 col/cycle, same rate f32r ran
    but without the PE transpose overhead);
  - softmax over the 4 modalities: exp on ACT straight out of PSUM;
    denominator adds on Pool (gpsimd); numerator products into separate
    tiles on DVE (no WAR stall against the Pool reads of e); reciprocal +
    final scaled multiply on DVE with the zero-modality rescale folded in;
  - per-segment emission order keeps next-tile casts ahead of this tile's
    exps in the ACT queue, and next-tile loads ahead of the XBAR transposes
    in the SP queue, so the PE never waits on stationaries.
"""

import sys

sys.path.insert(0, "/opt/trn_rl_repo")

from contextlib import ExitStack

import numpy as np

import concourse.bass as bass
import concourse.bacc as bacc
import concourse.mybir as mybir
import concourse.tile as tile
from concourse.bass_utils import run_bass_kernel_spmd

F32 = mybir.dt.float32
BF16 = mybir.dt.bfloat16
AF = mybir.ActivationFunctionType

N_CORES = 8
B_FULL = 16384
L = 1024
P = 128
B_SHARD = B_FULL // N_CORES          # 2048
NPT = B_SHARD // P                   # 16 patient tiles per core
NM = 4                               # modalities
NLC = L // P                         # 8 l-chunks (contraction)
NH = 2                               # k halves
KH = L // NH                         # 512

_CACHE: dict = {}


def _build(repeat: int = 1, *, elem: bool = True):
    nc = bacc.Bacc("TRN2", target_bir_lowering=False, debug=False)
    mods_d = [
        nc.dram_tensor(f"mod{m}", [B_SHARD, L], F32, kind="ExternalInput").ap()
        for m in range(NM)
    ]
    w_d = nc.dram_tensor("W", [L, L], F32, kind="ExternalInput").ap()
    out_d = nc.dram_tensor("out", [B_SHARD, L], F32, kind="ExternalOutput").ap()

    with tile.TileContext(nc) as tc, ExitStack() as ctx:
        wt_p = ctx.enter_context(tc.tile_pool(name="wt", bufs=1))
        wload_p = ctx.enter_context(tc.tile_pool(name="wload", bufs=2))
        mod_p = ctx.enter_context(tc.tile_pool(name="mod", bufs=4))
        modb_p = ctx.enter_context(tc.tile_pool(name="modb", bufs=2))
        modt_p = ctx.enter_context(tc.tile_pool(name="modt", bufs=2))
        e_p = ctx.enter_context(tc.tile_pool(name="e", bufs=4))
        n_p = ctx.enter_context(tc.tile_pool(name="n", bufs=2))
        rs_p = ctx.enter_context(tc.tile_pool(name="rs", bufs=4))
        tmp_p = ctx.enter_context(tc.tile_pool(name="tmp", bufs=2))
        out_p = ctx.enter_context(tc.tile_pool(name="outp", bufs=2))
        ps_q = ctx.enter_context(
            tc.tile_pool(name="ps_q", bufs=4, space=bass.MemorySpace.PSUM)
        )

        # ---- WT resident in SBUF (bf16): wt[p, j, k] = W[k, j*128 + p] ----
        wt = wt_p.tile([P, NLC, L], BF16, tag="wt")
        for kc in range(NLC):
            wk = wload_p.tile([P, L], F32, tag="wk")
            nc.sync.dma_start(wk[:], w_d[kc * P : (kc + 1) * P, :])
            wkb = wload_p.tile([P, L], BF16, tag="wkb")
            nc.scalar.copy(wkb[:], wk[:])
            nc.sync.dma_start_transpose(wt[:, :, kc * P : (kc + 1) * P], wkb[:])

        # ---------------- main loop, software-pipelined ----------------
        rep_cm = (
            tc.For_i(
                0,
                repeat,
                1,
                hint_engines=(
                    mybir.EngineType.PE,
                    mybir.EngineType.DVE,
                    mybir.EngineType.Activation,
                    mybir.EngineType.Pool,
                    mybir.EngineType.SP,
                ),
            )
            if repeat > 1
            else None
        )
        if rep_cm is not None:
            rep_cm.__enter__()

        def emit_load(p):
            row = slice(p * P, (p + 1) * P)
            mods = []
            for m in range(NM):
                mt = mod_p.tile([P, L], F32, tag=f"mod{m}")
                nc.sync.dma_start(mt[:], mods_d[m][row, :])
                mods.append(mt)
            return mods

        def emit_conv(p, mods):
            """f32 -> bf16 casts on ACT; accum_out rides along to produce the
            per-modality row sums (zero-modality detection) for free."""
            rsum = rs_p.tile([P, NM], F32, tag="rsum")
            modbs = []
            for m in range(NM):
                mb = modb_p.tile([P, L], BF16, tag=f"modb{m}")
                nc.scalar.activation(
                    mb[:], mods[m][:], AF.Copy, accum_out=rsum[:, m : m + 1]
                )
                modbs.append(mb)
            return modbs, rsum

        def emit_transp(p, modbs):
            modts = []
            for m in range(NM):
                mT = modt_p.tile([P, NLC, P], BF16, tag=f"modt{m}")
                nc.sync.dma_start_transpose(mT[:], modbs[m][:])
                modts.append(mT)
            return modts

        def emit_pe(p, modts):
            """Score matmuls + trailing exps for tile p."""
            es = {}
            for m in range(NM):
                sqs = []
                for _h in range(NH):
                    sq = ps_q.tile([P, KH], F32, tag="sq")
                    sqs.append(sq)
                for j in range(NLC):
                    for h in range(NH):
                        nc.tensor.matmul(
                            sqs[h][:],
                            modts[m][:, j, :],
                            wt[:, j, h * KH : (h + 1) * KH],
                            start=(j == 0),
                            stop=(j == NLC - 1),
                        )
                if elem:
                    for h in range(NH):
                        e = e_p.tile([P, KH], F32, tag=f"e{m}")
                        nc.scalar.activation(e[:], sqs[h][:], AF.Exp)
                        es[(m, h)] = e
            return es

        def emit_tail(state):
            """Softmax combine + output for tile p (lags one segment)."""
            p, mods, es, rsum = state
            row = slice(p * P, (p + 1) * P)
            zt = tmp_p.tile([P, NM], F32, tag="zt")
            zs = tmp_p.tile([P, 1], F32, tag="zs")
            nc.vector.tensor_scalar(
                out=zt[:],
                in0=rsum[:],
                scalar1=0.0,
                scalar2=None,
                op0=mybir.AluOpType.is_equal,
                op1=mybir.AluOpType.add,
                accum_out=zs[:],
            )
            scaler = tmp_p.tile([P, 1], F32, tag="scaler")
            nc.vector.tensor_scalar_add(scaler[:], zs[:], 1.0)
            ot = out_p.tile([P, L], F32, tag="ot")
            for h in range(NH):
                e0, e1, e2, e3 = (es[(m, h)] for m in range(NM))
                # denominator on Pool; reads e_m, no write-back into them
                d01 = tmp_p.tile([P, KH], F32, tag="d01")
                d23 = tmp_p.tile([P, KH], F32, tag="d23")
                nc.gpsimd.tensor_add(d01[:], e0[:], e1[:])
                nc.gpsimd.tensor_add(d23[:], e2[:], e3[:])
                nc.gpsimd.tensor_add(d01[:], d01[:], d23[:])
                # numerator into separate tiles on DVE (no WAR vs Pool reads)
                ns = []
                for m in range(NM):
                    nm_t = n_p.tile([P, KH], F32, tag=f"n{m}")
                    nc.vector.tensor_mul(
                        nm_t[:],
                        es[(m, h)][:],
                        mods[m][:, h * KH : (h + 1) * KH],
                    )
                    ns.append(nm_t)
                nc.vector.tensor_add(ns[0][:], ns[0][:], ns[1][:])
                nc.gpsimd.tensor_add(ns[2][:], ns[2][:], ns[3][:])
                nc.vector.reciprocal_approx_fast(out=d01[:], in_=d01[:])
                nc.vector.tensor_add(ns[0][:], ns[0][:], ns[2][:])
                # ot = (r * scaler) * num in one DVE op
                nc.vector.scalar_tensor_tensor(
                    out=ot[:, h * KH : (h + 1) * KH],
                    in0=d01[:],
                    scalar=scaler[:],
                    in1=ns[0][:],
                    op0=mybir.AluOpType.mult,
                    op1=mybir.AluOpType.mult,
                )
            nc.sync.dma_start(out_d[row, :], ot[:])

        # prologue
        loaded = {0: emit_load(0)}
        conv = {0: emit_conv(0, loaded[0])}
        transposed = {0: emit_transp(0, conv[0][0])}
        if NPT > 1:
            loaded[1] = emit_load(1)

        prev = None
        for p in range(NPT):
            if p + 2 < NPT:
                loaded[p + 2] = emit_load(p + 2)
            if p + 1 < NPT:
                conv[p + 1] = emit_conv(p + 1, loaded[p + 1])
                transposed[p + 1] = emit_transp(p + 1, conv[p + 1][0])
            es = emit_pe(p, transposed.pop(p))
            state = (p, loaded[p], es, conv[p][1])
            if prev is not None and elem:
                emit_tail(prev)
            prev = state
        if elem:
            emit_tail(prev)

        if rep_cm is not None:
            rep_cm.__exit__(None, None, None)

    nc.compile()
    return nc


def _get_nc(repeat: int = 1, **flags):
    key = ("nc", repeat, tuple(sorted(flags.items())))
    if key not in _CACHE:
        _CACHE[key] = _build(repeat, **flags)
    return _CACHE[key]


def _run(inputs, trace=False):
    nc = _get_nc()
    w = np.ascontiguousarray(np.asarray(inputs["W"], dtype=np.float32))
    in_maps = []
    for c in range(N_CORES):
        sl = slice(c * B_SHARD, (c + 1) * B_SHARD)
        im = {"W": w}
        for m in range(NM):
            im[f"mod{m}"] = np.ascontiguousarray(
                np.asarray(inputs[f"mod{m}"], dtype=np.float32)[sl]
            )
        in_maps.append(im)
    return run_bass_kernel_spmd(
        nc, in_maps, core_ids=list(range(N_CORES)), trace=trace
    )


def kernel(**inputs) -> np.ndarray:
    res = _run(inputs, trace=False)
    return np.concatenate(
        [res.results[c]["out"] for c in range(N_CORES)], axis=0
    ).astype(np.float32)
